# revision 2
# baseline (speedup 1.0000x reference)
"""Trainium2 Bass kernel for the intra-batch point-cloud contrastive loss.

Math (matches the reference exactly):
  feats   = features_in.reshape(C, M).T    (row-major reinterpret), M = B*N
  labels  = labels_in.reshape(-1)
  sel     = bernoulli(key 42, min(750/(count+1),1)[labels])   (host, jax CPU)
  nv      = feats / ||feats||
  dp      = exp(nv @ nv.T / TEMP), diagonal zeroed
  pos_i   = sum_{j sel, same class} dp_ij ; neg over different class
  loss    = mean over selected i of -log(pos/(pos+neg))

Only selected points contribute (unselected rows give 0 loss, unselected
columns have zero weight), so the device works on the compacted point set
(~37% of M).  Columns are sharded over 8 cores.  To keep the program
SPMD-identical, each core receives the compacted feature matrix *rolled*
so its own columns sit first; the diagonal then lands at a fixed position
for every core and is zeroed with one shared (1-eye) mask.

Per core (L = M_pad/8 local columns, nT = M_pad/128 row chunks):
  mm1 (PE):  G_t = nvT[:, chunk t].T @ nvT[:, :L]     [128, L] fp32 psum
  exp (ACT): dp_t = exp(G_t / TEMP)                    -> SBUF
  mask(DVE): zero the diagonal sub-block (t < L/128)
  mm2 (PE):  S += W_t.T @ dp_t   (W = sel*onehot(label), [4, L] psum accum)
The host gathers the per-core S blocks and finishes the O(n_sel) epilogue.
"""

import numpy as np

TEMP = 0.07
NUM_CLASSES = 4
N_CORES = 8
P = 128

_NEFF_CACHE = {}


def _compute_sel(labels_flat):
    """Selection mask, bit-exact with the reference (jax threefry, key 42)."""
    import jax
    import jax.numpy as jnp

    cpu = jax.devices("cpu")[0]
    with jax.default_device(cpu):
        lab_j = jnp.asarray(labels_flat)
        counts = jnp.bincount(lab_j, length=NUM_CLASSES)
        keep_p = jnp.minimum(750.0 / (counts.astype(jnp.float32) + 1.0), 1.0)
        p = keep_p[lab_j]
        sel = jax.random.bernoulli(jax.random.key(42), p)
        return np.asarray(sel)


def _build_kernel(M_pad):
    import concourse.bass as bass
    import concourse.mybir as mybir
    import concourse.tile as tile

    L = M_pad // N_CORES          # local columns per core
    nT = M_pad // P               # 128-row chunks
    nL = L // P                   # chunks containing this core's diagonal
    f32 = mybir.dt.float32

    # consts layout (single tensor -> single DMA -> single semaphore):
    # cols [0, nT*4)                     W chunks (mm2 lhsT)
    # cols [nT*4, nT*4+P)                128x128 identity
    # cols [nT*4+P, nT*4+P+2L-P)         dwide (-1e9 shifted diagonal)
    CW = nT * NUM_CLASSES + P + (2 * L - P)
    o_eye = nT * NUM_CLASSES
    o_dw = o_eye + P

    nc = bass.Bass()
    packed_d = nc.dram_tensor("packed", [P, M_pad + CW], f32, kind="ExternalInput")
    s_d = nc.dram_tensor("s_out", [NUM_CLASSES, L], f32, kind="ExternalOutput")

    with tile.TileContext(nc) as tc:
        with (
            tc.tile_pool(name="singles", bufs=1) as singles,
            tc.tile_pool(name="dp", bufs=nT) as dp_pool,
            tc.tile_pool(name="ps", bufs=7, space="PSUM") as ps_pool,
            tc.tile_pool(name="acc", bufs=1, space="PSUM") as acc_pool,
        ):
            packed = singles.tile([P, M_pad + CW], f32)
            # ONE SWDGE (gpsimd) DMA -> one completion semaphore.  Several
            # DMAs (or an HWDGE multi-queue fan-out) would attach more inline
            # sync waits than this walrus build allows per instruction.
            nc.gpsimd.dma_start(out=packed[:], in_=packed_d[:])
            nvt = packed[0:64, 0:M_pad]
            w_sb = packed[:, M_pad + 0:M_pad + o_eye]
            eye_sb = packed[:, M_pad + o_eye:M_pad + o_dw]
            dwide_sb = packed[:, M_pad + o_dw:M_pad + CW]

            s_ps = acc_pool.tile([NUM_CLASSES, L], f32)
            rhs = nvt[:, 0:L]
            # dwide[p, c] = -1e9 iff c == p + (nL-1)*P; sliced so the -1e9
            # diagonal lands on this chunk's own columns [t*P, t*P+P).
            off0 = (nL - 1) * P
            for t in range(nT):
                ps = ps_pool.tile([P, L], f32)
                nc.tensor.matmul(
                    ps[:], nvt[:, t * P:(t + 1) * P], rhs,
                    start=True, stop=(t >= nL),
                )
                if t < nL:
                    # G += I.T @ D = D: pushes the diagonal to -1e9 so that
                    # exp() maps it to exactly 0.
                    nc.tensor.matmul(
                        ps[:], eye_sb, dwide_sb[:, off0 - t * P: off0 - t * P + L],
                        start=False, stop=True,
                    )
                dp = dp_pool.tile([P, L], f32)
                nc.scalar.activation(
                    dp[:], ps[:], mybir.ActivationFunctionType.Exp,
                    scale=float(1.0 / TEMP),
                )
                nc.tensor.matmul(
                    s_ps[:], w_sb[:, t * NUM_CLASSES:(t + 1) * NUM_CLASSES], dp[:],
                    start=(t == 0), stop=(t == nT - 1),
                )

            s_sb = singles.tile([NUM_CLASSES, L], f32)
            nc.scalar.copy(s_sb[:], s_ps[:])
            nc.gpsimd.dma_start(out=s_d[:], in_=s_sb[:])

    _split_multi_waits(nc)
    return nc


def _split_multi_waits(nc):
    """Walrus in this toolchain accepts only one inline sync-wait per
    instruction.  Tile's kernel-tail drain aggregates one wait per live
    semaphore, so hoist all but the last wait onto same-engine nops."""
    import concourse.mybir as mybir

    for fn in nc.m.functions:
        for blk in fn.blocks:
            insts = list(blk.instructions)
            out = []
            for inst in insts:
                si = inst.sync_info
                waits = list(si.on_wait) if si is not None and si.on_wait else []
                if len(waits) > 1:
                    for w in waits[:-1]:
                        out.append(mybir.InstNoOp(
                            name=nc.get_next_instruction_name(),
                            engine=inst.engine,
                            bass_nofuse=True,
                            sync_info=mybir.SyncInfo(on_wait=[w], on_update=[]),
                        ))
                    si.on_wait = waits[-1:]
                out.append(inst)
            if len(out) != len(insts):
                blk.instructions = out


def _get_kernel(M_pad):
    if M_pad not in _NEFF_CACHE:
        _NEFF_CACHE[M_pad] = _build_kernel(M_pad)
    return _NEFF_CACHE[M_pad]


_results = [None]


def kernel(features_in, labels_in, _trace=False, _results=_results):
    from concourse.bass_utils import run_bass_kernel_spmd

    features_in = np.asarray(features_in, dtype=np.float32)
    B, C, N = features_in.shape
    M = B * N
    labels = np.asarray(labels_in).reshape(-1).astype(np.int64)

    fT = features_in.reshape(C, M)                      # [C, M] reinterpret
    sel = _compute_sel(labels)
    idx = np.nonzero(sel)[0]
    n_sel = int(idx.size)
    n_div = max(n_sel, 1)

    norms = np.sqrt(np.sum(fT * fT, axis=0, dtype=np.float32)).astype(np.float32)
    nvT = (fT / norms).astype(np.float32)

    lab_sel = labels[idx]
    per_core = N_CORES * P
    M_pad = max(((n_sel + per_core - 1) // per_core) * per_core, per_core)
    L = M_pad // N_CORES
    nT = M_pad // P

    nvT_pad = np.zeros((C, M_pad), np.float32)
    nvT_pad[:, :n_sel] = nvT[:, idx]
    W = np.zeros((M_pad, NUM_CLASSES), np.float32)
    W[np.arange(n_sel), lab_sel] = 1.0

    nL = L // P
    eye = np.eye(P, dtype=np.float32)
    dwide = np.zeros((P, 2 * L - P), np.float32)
    dwide[np.arange(P), np.arange(P) + (nL - 1) * P] = -1e9

    in_maps = []
    for k in range(N_CORES):
        nv_k = np.ascontiguousarray(np.roll(nvT_pad, -L * k, axis=1))
        W_k = np.roll(W, -L * k, axis=0)
        # lhsT chunk t lives at columns [4t, 4t+4): w_arr[p, 4t+c] = W_k[128t+p, c]
        w_arr = W_k.reshape(nT, P, NUM_CLASSES).transpose(1, 0, 2).reshape(
            P, nT * NUM_CLASSES
        )
        consts = np.concatenate([w_arr, eye, dwide], axis=1)
        packed = np.zeros((P, M_pad + consts.shape[1]), np.float32)
        packed[:C, :M_pad] = nv_k
        packed[:, M_pad:] = consts
        in_maps.append({"packed": packed})

    nc = _get_kernel(M_pad)
    res = run_bass_kernel_spmd(nc, in_maps, core_ids=list(range(N_CORES)),
                               trace=_trace)
    _results[0] = res

    S = np.concatenate([res.results[k]["s_out"] for k in range(N_CORES)], axis=1)
    S = S[:, :n_sel]
    denom = np.sum(S, axis=0, dtype=np.float32).astype(np.float32)
    numer = S[lab_sel, np.arange(n_sel)]
    per = (-np.log(numer / denom)).astype(np.float32)
    loss = np.float32(per.sum(dtype=np.float32) / np.float32(n_div))
    return np.asarray(loss, dtype=np.float32)



# revision 4
# speedup vs baseline: 2.1329x; 2.1329x over previous
"""Trainium2 Bass kernel for the intra-batch point-cloud contrastive loss.

Math (matches the reference):
  feats   = features_in.reshape(C, M).T    (row-major reinterpret), M = B*N
  labels  = labels_in.reshape(-1)
  sel     = bernoulli(key 42, min(750/(count+1),1)[labels])   (host, jax CPU)
  nv      = feats / ||feats||
  dp      = exp(nv @ nv.T / TEMP), diagonal zeroed
  pos_i   = sum_{j sel, same class} dp_ij ; neg over different class
  loss    = mean over selected i of -log(pos/(pos+neg))

Only selected points matter (~3001 of 8192).  The selected points are
SORTED BY CLASS and each class is padded with zero-feature points to
2*SEG columns (SEG=384 -> M_pad=3072).  Rows are sharded over 8 cores
(L = SEG rows each); each core computes its row-block of the similarity
matrix against ALL columns in bf16, exponentiates, and reduces each row
over the 8 column segments of SEG.  Because columns are class-sorted and
the per-core column roll is a multiple of SEG, every segment is
class-pure, so the 8 per-segment row sums are exactly the per-class
sums (host maps segment -> class per core, subtracts the exp(0)=1
contribution of the zero pads, and finishes the tiny O(n_sel) epilogue).

Per core (nL = SEG/128 row chunks, 8 column segments):
  mm1 (PE):  ps[s] = nv[:, rP:(r+1)P].T @ nv[:, s*SEG:(s+1)*SEG]  bf16
  diag (PE): ps[0] += I.T @ dwide  (-1e9 on the rolled diagonal)
  exp (ACT): dp = exp(ps / TEMP) -> SBUF bf16   (two [128,4,SEG] instrs)
  sum (DVE): TT-add tree SEG->SEG/2->SEG/4, then tensor_reduce -> [128,8] f32
The kernel returns [128, nL*8] per-row segment sums; no second matmul
chain and no O(M^2) output traffic.
"""

import numpy as np

TEMP = 0.07
NUM_CLASSES = 4
N_CORES = 8
P = 128

_NEFF_CACHE = {}
_results = [None]


def _compute_sel(labels_flat):
    """Selection mask, bit-exact with the reference (jax threefry, key 42)."""
    import jax
    import jax.numpy as jnp

    cpu = jax.devices("cpu")[0]
    with jax.default_device(cpu):
        lab_j = jnp.asarray(labels_flat)
        counts = jnp.bincount(lab_j, length=NUM_CLASSES)
        keep_p = jnp.minimum(750.0 / (counts.astype(jnp.float32) + 1.0), 1.0)
        p = keep_p[lab_j]
        sel = jax.random.bernoulli(jax.random.key(42), p)
        return np.asarray(sel)


def _build_kernel(SEG):
    import concourse.bass as bass
    import concourse.mybir as mybir
    import concourse.tile as tile

    nL = SEG // P                 # row chunks per core
    M_pad = 8 * SEG
    H = M_pad // 2                # columns per input half
    DW = 2 * SEG - P              # dwide width
    f32 = mybir.dt.float32
    bf16 = mybir.dt.bfloat16
    Exp = mybir.ActivationFunctionType.Exp
    add = mybir.AluOpType.add
    AX = mybir.AxisListType.X

    nc = bass.Bass()
    nva_d = nc.dram_tensor("nva", [64, H], bf16, kind="ExternalInput")
    nvb_d = nc.dram_tensor("nvb", [64, H], bf16, kind="ExternalInput")
    consts_d = nc.dram_tensor("consts", [P, P + DW], bf16, kind="ExternalInput")
    acc_d = nc.dram_tensor("acc", [P, nL * 8], f32, kind="ExternalOutput")

    with tile.TileContext(nc) as tc:
        with (
            tc.tile_pool(name="singles", bufs=1) as singles,
            tc.tile_pool(name="dp_pool", bufs=2) as dp_pool,
            tc.tile_pool(name="t1_pool", bufs=2) as t1_pool,
            tc.tile_pool(name="t2_pool", bufs=2) as t2_pool,
            tc.tile_pool(name="ps_pool", bufs=2, space="PSUM") as ps_pool,
        ):
            nva = singles.tile([64, H], bf16)
            nvb = singles.tile([64, H], bf16)
            consts = singles.tile([P, P + DW], bf16)
            # Three parallel DMA queues: 2 HWDGE (sync/scalar) + SWDGE.
            nc.sync.dma_start(out=nva[:], in_=nva_d[:])
            nc.scalar.dma_start(out=nvb[:], in_=nvb_d[:])
            nc.gpsimd.dma_start(out=consts[:], in_=consts_d[:])
            eye = consts[:, 0:P]
            dwide = consts[:, P:P + DW]

            acc = singles.tile([P, nL, 8], f32)

            for r in range(nL):
                stat = nva[:, r * P:(r + 1) * P]
                dp = dp_pool.tile([P, 8, SEG], bf16)
                for h in range(2):
                    src = nva if h == 0 else nvb
                    ps = ps_pool.tile([P, 4, 512], f32)
                    if h == 0:
                        # dwide[p, c] = -1e9 iff c == p + (nL-1)*P; slice so
                        # it lands on (p, r*P + p): exp maps the diagonal to 0.
                        off = (nL - 1 - r) * P
                        nc.tensor.matmul(
                            ps[:, 0, 0:SEG], eye, dwide[:, off:off + SEG],
                            start=True, stop=False,
                        )
                    for s4 in range(4):
                        nc.tensor.matmul(
                            ps[:, s4, 0:SEG], stat,
                            src[:, s4 * SEG:(s4 + 1) * SEG],
                            start=not (h == 0 and s4 == 0),
                            stop=True,
                        )
                    nc.scalar.activation(
                        dp[:, 4 * h:4 * h + 4, :], ps[:, :, 0:SEG],
                        Exp, scale=float(1.0 / TEMP),
                    )
                t1 = t1_pool.tile([P, 8, SEG // 2], bf16)
                for h in range(2):
                    nc.vector.tensor_tensor(
                        t1[:, 4 * h:4 * h + 4, :],
                        dp[:, 4 * h:4 * h + 4, 0:SEG // 2],
                        dp[:, 4 * h:4 * h + 4, SEG // 2:SEG],
                        op=add,
                    )
                t2 = t2_pool.tile([P, 8, SEG // 4], bf16)
                nc.vector.tensor_tensor(
                    t2[:], t1[:, :, 0:SEG // 4], t1[:, :, SEG // 4:SEG // 2],
                    op=add,
                )
                nc.vector.tensor_reduce(acc[:, r, :], t2[:], axis=AX, op=add)

            nc.sync.dma_start(out=acc_d[:], in_=acc[:])

    _split_multi_waits(nc)
    return nc


def _split_multi_waits(nc):
    """Walrus in this toolchain accepts only one inline sync-wait per
    instruction.  Tile's kernel-tail drain aggregates one wait per live
    semaphore, so hoist all but the last wait onto same-engine nops."""
    import concourse.mybir as mybir

    for fn in nc.m.functions:
        for blk in fn.blocks:
            insts = list(blk.instructions)
            out = []
            for inst in insts:
                si = inst.sync_info
                waits = list(si.on_wait) if si is not None and si.on_wait else []
                if len(waits) > 1:
                    for w in waits[:-1]:
                        out.append(mybir.InstNoOp(
                            name=nc.get_next_instruction_name(),
                            engine=inst.engine,
                            bass_nofuse=True,
                            sync_info=mybir.SyncInfo(on_wait=[w], on_update=[]),
                        ))
                    si.on_wait = waits[-1:]
                out.append(inst)
            if len(out) != len(insts):
                blk.instructions = out


def _get_kernel(SEG):
    if SEG not in _NEFF_CACHE:
        _NEFF_CACHE[SEG] = _build_kernel(SEG)
    return _NEFF_CACHE[SEG]


def kernel(features_in, labels_in, _trace=False, _results=_results):
    import ml_dtypes
    from concourse.bass_utils import run_bass_kernel_spmd

    features_in = np.asarray(features_in, dtype=np.float32)
    B, C, N = features_in.shape
    M = B * N
    labels = np.asarray(labels_in).reshape(-1).astype(np.int64)

    fT = features_in.reshape(C, M)                      # [C, M] reinterpret
    sel = _compute_sel(labels)
    idx = np.nonzero(sel)[0]
    n_sel = int(idx.size)
    lab_sel = labels[idx]

    norms = np.sqrt(np.sum(fT * fT, axis=0, dtype=np.float32)).astype(np.float32)
    nvT = (fT / norms).astype(np.float32)

    # Sort selected points by class; pad each class block to 2*SEG columns.
    n_c = np.bincount(lab_sel, minlength=NUM_CLASSES)
    SEG = max(384, 128 * int(np.ceil(n_c.max() / 256.0)))
    CAP = 2 * SEG                 # per-class capacity
    M_pad = 8 * SEG
    H = M_pad // 2
    nL = SEG // P

    order = np.argsort(lab_sel, kind="stable")
    G = np.zeros((64, M_pad), dtype=ml_dtypes.bfloat16)
    # position of each sorted point in the padded layout
    pos = np.concatenate(
        [np.arange(n_c[c]) + CAP * c for c in range(NUM_CLASSES)]
    )
    nv_sel = nvT[:, idx[order]].astype(ml_dtypes.bfloat16)
    G[:, pos] = nv_sel

    eye = np.eye(P, dtype=ml_dtypes.bfloat16)
    dwide = np.zeros((P, 2 * SEG - P), dtype=ml_dtypes.bfloat16)
    dwide[np.arange(P), np.arange(P) + (nL - 1) * P] = -1e9
    consts = np.concatenate([eye, dwide], axis=1)

    in_maps = []
    for k in range(N_CORES):
        nv_k = np.roll(G, -SEG * k, axis=1)
        in_maps.append({
            "nva": np.ascontiguousarray(nv_k[:, :H]),
            "nvb": np.ascontiguousarray(nv_k[:, H:]),
            "consts": consts,
        })

    nc = _get_kernel(SEG)
    res = run_bass_kernel_spmd(nc, in_maps, core_ids=list(range(N_CORES)),
                               trace=_trace)
    _results[0] = res

    # acc[k][p, r*8+s]: row sum of point (SEG*k + P*r + p) over local col
    # segment s = global segment (s+k) % 8.
    S_glob = np.zeros((M_pad, 8), dtype=np.float64)
    for k in range(N_CORES):
        a = np.asarray(res.results[k]["acc"], dtype=np.float64)
        a = a.reshape(P, nL, 8).transpose(1, 0, 2).reshape(SEG, 8)
        S_glob[SEG * k:SEG * (k + 1), (np.arange(8) + k) % 8] = a

    S4 = S_glob.reshape(M_pad, NUM_CLASSES, 2).sum(axis=2)  # [M_pad, 4]
    pads = (CAP - n_c).astype(np.float64)                   # exp(0)=1 per pad
    Sreal = S4[pos] - pads[None, :]                         # [n_sel, 4] sorted
    lab_sorted = lab_sel[order]
    numer = Sreal[np.arange(n_sel), lab_sorted]
    denom = Sreal.sum(axis=1)
    per = -np.log(numer / denom)
    loss = np.float32(per.sum() / max(n_sel, 1))
    return np.asarray(loss, dtype=np.float32)


# revision 5
# speedup vs baseline: 2.2314x; 1.0462x over previous
"""Trainium2 Bass kernel for the intra-batch point-cloud contrastive loss.

Math (matches the reference):
  feats   = features_in.reshape(C, M).T    (row-major reinterpret), M = B*N
  labels  = labels_in.reshape(-1)
  sel     = bernoulli(key 42, min(750/(count+1),1)[labels])   (host, jax CPU)
  nv      = feats / ||feats||
  dp      = exp(nv @ nv.T / TEMP), diagonal zeroed
  pos_i   = sum_{j sel, same class} dp_ij ; neg over different class
  loss    = mean over selected i of -log(pos/(pos+neg))

Only selected points matter (~3001 of 8192).  The selected points are
SORTED BY CLASS and each class is padded with zero-feature points to
2*SEG columns (SEG=384 -> M_pad=3072).  Rows are sharded over 8 cores
(SEG rows each, rolled so each core's own columns come first); each core
computes its row-block of the similarity matrix against ALL columns in
bf16, exponentiates, and reduces each row over the 8 column segments of
SEG.  Columns are class-sorted and the per-core roll is a multiple of
SEG, so every segment is class-pure: the 8 per-segment row sums ARE the
per-class sums.  The host maps segment -> class per core, subtracts the
exp(0)=1 contribution of the zero pads, and runs the tiny O(n_sel)
epilogue.

Per core and row chunk r (nL = SEG/128 chunks):
  mm1 (PE):  ps[512b:512b+512] = nv[:, rP:(r+1)P].T @ nv[:, cols]   bf16
  diag (PE): ps[rP:rP+128] += I.T @ (-1e9*I)   (kills the diagonal)
  exp (ACT): dp = exp(ps / TEMP) -> SBUF bf16  (two [128, 1536] instrs)
  sum (DVE): TT-add fold 384->192->96, tensor_reduce -> acc[128, 8] f32
No second matmul chain, no O(M^2) output traffic.
"""

import numpy as np

TEMP = 0.07
NUM_CLASSES = 4
N_CORES = 8
P = 128

_NEFF_CACHE = {}
_results = [None]


def _compute_sel(labels_flat):
    """Selection mask, bit-exact with the reference (jax threefry, key 42)."""
    import jax
    import jax.numpy as jnp

    cpu = jax.devices("cpu")[0]
    with jax.default_device(cpu):
        lab_j = jnp.asarray(labels_flat)
        counts = jnp.bincount(lab_j, length=NUM_CLASSES)
        keep_p = jnp.minimum(750.0 / (counts.astype(jnp.float32) + 1.0), 1.0)
        p = keep_p[lab_j]
        sel = jax.random.bernoulli(jax.random.key(42), p)
        return np.asarray(sel)


def _build_kernel(SEG):
    import concourse.bass as bass
    import concourse.mybir as mybir
    import concourse.tile as tile

    nL = SEG // P                 # row chunks per core
    M_pad = 8 * SEG
    HB = M_pad // 2               # bytes of columns per half (h0: nv0-2, h1: nvb)
    f32 = mybir.dt.float32
    bf16 = mybir.dt.bfloat16
    Exp = mybir.ActivationFunctionType.Exp
    add = mybir.AluOpType.add
    AX = mybir.AxisListType.X
    NB = HB // 512                # 512-col blocks per half (3 for SEG=384)

    nc = bass.Bass()
    nv_d = [
        nc.dram_tensor(f"nv{i}", [64, 512], bf16, kind="ExternalInput")
        for i in range(NB)
    ]
    nvb_d = nc.dram_tensor("nvb", [64, HB], bf16, kind="ExternalInput")
    consts_d = nc.dram_tensor("consts", [P, 2 * P], bf16, kind="ExternalInput")
    acc_d = nc.dram_tensor("acc", [P, nL * 8], f32, kind="ExternalOutput")

    with tile.TileContext(nc) as tc:
        with (
            tc.tile_pool(name="singles", bufs=1) as singles,
            tc.tile_pool(name="dp_pool", bufs=2) as dp_pool,
            tc.tile_pool(name="t1_pool", bufs=2) as t1_pool,
            tc.tile_pool(name="t2_pool", bufs=2) as t2_pool,
            tc.tile_pool(name="ps_pool", bufs=2, space="PSUM") as ps_pool,
        ):
            nva = [singles.tile([64, 512], bf16, name=f"nva{i}") for i in range(NB)]
            nvb = singles.tile([64, HB], bf16)
            consts = singles.tile([P, 2 * P], bf16)
            # All HWDGE queues; ordered so the first matmuls' data lands first.
            for i in range(NB):
                nc.sync.dma_start(out=nva[i][:], in_=nv_d[i][:])
            nc.sync.dma_start(out=consts[:], in_=consts_d[:])
            nc.scalar.dma_start(out=nvb[:], in_=nvb_d[:])
            eye = consts[:, 0:P]
            eyeneg = consts[:, P:2 * P]

            acc = singles.tile([P, nL, 8], f32)

            for r in range(nL):
                stat = nva[0][:, r * P:(r + 1) * P]
                dp = dp_pool.tile([P, 8, SEG], bf16)
                for h in range(2):
                    ps = ps_pool.tile([P, HB], f32)
                    for b in range(NB):
                        src = nva[b] if h == 0 else nvb[:, 512 * b:512 * (b + 1)]
                        nc.tensor.matmul(
                            ps[:, 512 * b:512 * (b + 1)], stat, src,
                            start=True, stop=not (h == 0 and b == 0),
                        )
                    if h == 0:
                        # add -1e9 on the rolled diagonal (cols rP..rP+P of
                        # block 0) so exp maps it to exactly 0
                        nc.tensor.matmul(
                            ps[:, r * P:(r + 1) * P], eye, eyeneg,
                            start=False, stop=True,
                        )
                    nc.scalar.activation(
                        dp[:, 4 * h:4 * h + 4, :], ps[:],
                        Exp, scale=float(1.0 / TEMP),
                    )
                t1 = t1_pool.tile([P, 8, SEG // 2], bf16)
                t2 = t2_pool.tile([P, 8, SEG // 4], bf16)
                if r < nL - 1:
                    for h in range(2):
                        nc.vector.tensor_tensor(
                            t1[:, 4 * h:4 * h + 4, :],
                            dp[:, 4 * h:4 * h + 4, 0:SEG // 2],
                            dp[:, 4 * h:4 * h + 4, SEG // 2:SEG],
                            op=add,
                        )
                    nc.vector.tensor_tensor(
                        t2[:], t1[:, :, 0:SEG // 4], t1[:, :, SEG // 4:SEG // 2],
                        op=add,
                    )
                    nc.vector.tensor_reduce(acc[:, r, :], t2[:], axis=AX, op=add)
                else:
                    # last chunk: finish each half independently so the h0
                    # reduce overlaps the h1 exp (shorter serial tail)
                    for h in range(2):
                        sl = slice(4 * h, 4 * h + 4)
                        nc.vector.tensor_tensor(
                            t1[:, sl, :],
                            dp[:, sl, 0:SEG // 2], dp[:, sl, SEG // 2:SEG],
                            op=add,
                        )
                        nc.vector.tensor_tensor(
                            t2[:, sl, :],
                            t1[:, sl, 0:SEG // 4], t1[:, sl, SEG // 4:SEG // 2],
                            op=add,
                        )
                        nc.vector.tensor_reduce(
                            acc[:, r, sl], t2[:, sl, :], axis=AX, op=add,
                        )

            nc.sync.dma_start(out=acc_d[:], in_=acc[:])

    _split_multi_waits(nc)
    return nc


def _split_multi_waits(nc):
    """Walrus in this toolchain accepts only one inline sync-wait per
    instruction.  Tile's kernel-tail drain aggregates one wait per live
    semaphore, so hoist all but the last wait onto same-engine nops."""
    import concourse.mybir as mybir

    for fn in nc.m.functions:
        for blk in fn.blocks:
            insts = list(blk.instructions)
            out = []
            for inst in insts:
                si = inst.sync_info
                waits = list(si.on_wait) if si is not None and si.on_wait else []
                if len(waits) > 1:
                    for w in waits[:-1]:
                        out.append(mybir.InstNoOp(
                            name=nc.get_next_instruction_name(),
                            engine=inst.engine,
                            bass_nofuse=True,
                            sync_info=mybir.SyncInfo(on_wait=[w], on_update=[]),
                        ))
                    si.on_wait = waits[-1:]
                out.append(inst)
            if len(out) != len(insts):
                blk.instructions = out


def _get_kernel(SEG):
    if SEG not in _NEFF_CACHE:
        _NEFF_CACHE[SEG] = _build_kernel(SEG)
    return _NEFF_CACHE[SEG]


def kernel(features_in, labels_in, _trace=False, _results=_results):
    import ml_dtypes
    from concourse.bass_utils import run_bass_kernel_spmd

    features_in = np.asarray(features_in, dtype=np.float32)
    B, C, N = features_in.shape
    M = B * N
    labels = np.asarray(labels_in).reshape(-1).astype(np.int64)

    fT = features_in.reshape(C, M)                      # [C, M] reinterpret
    sel = _compute_sel(labels)
    idx = np.nonzero(sel)[0]
    n_sel = int(idx.size)
    lab_sel = labels[idx]

    norms = np.sqrt(np.sum(fT * fT, axis=0, dtype=np.float32)).astype(np.float32)
    nvT = (fT / norms).astype(np.float32)

    # Sort selected points by class; pad each class block to 2*SEG columns.
    n_c = np.bincount(lab_sel, minlength=NUM_CLASSES)
    SEG = max(384, 128 * int(np.ceil(n_c.max() / 256.0)))
    CAP = 2 * SEG                 # per-class capacity
    M_pad = 8 * SEG
    HB = M_pad // 2
    NB = HB // 512

    order = np.argsort(lab_sel, kind="stable")
    G = np.zeros((64, M_pad), dtype=ml_dtypes.bfloat16)
    # position of each sorted point in the padded layout
    pos = np.concatenate(
        [np.arange(n_c[c]) + CAP * c for c in range(NUM_CLASSES)]
    )
    nv_sel = nvT[:, idx[order]].astype(ml_dtypes.bfloat16)
    G[:, pos] = nv_sel

    eye = np.eye(P, dtype=ml_dtypes.bfloat16)
    eyeneg = (np.eye(P, dtype=np.float32) * -1e9).astype(ml_dtypes.bfloat16)
    consts = np.concatenate([eye, eyeneg], axis=1)

    in_maps = []
    for k in range(N_CORES):
        nv_k = np.roll(G, -SEG * k, axis=1)
        m = {
            f"nv{i}": np.ascontiguousarray(nv_k[:, 512 * i:512 * (i + 1)])
            for i in range(NB)
        }
        m["nvb"] = np.ascontiguousarray(nv_k[:, HB:])
        m["consts"] = consts
        in_maps.append(m)

    nc = _get_kernel(SEG)
    res = run_bass_kernel_spmd(nc, in_maps, core_ids=list(range(N_CORES)),
                               trace=_trace)
    _results[0] = res

    nL = SEG // P
    # acc[k][p, r*8+s]: row sum of point (SEG*k + P*r + p) over local col
    # segment s = global segment (s+k) % 8.
    S_glob = np.zeros((M_pad, 8), dtype=np.float64)
    for k in range(N_CORES):
        a = np.asarray(res.results[k]["acc"], dtype=np.float64)
        a = a.reshape(P, nL, 8).transpose(1, 0, 2).reshape(SEG, 8)
        S_glob[SEG * k:SEG * (k + 1), (np.arange(8) + k) % 8] = a

    S4 = S_glob.reshape(M_pad, NUM_CLASSES, 2).sum(axis=2)  # [M_pad, 4]
    pads = (CAP - n_c).astype(np.float64)                   # exp(0)=1 per pad
    Sreal = S4[pos] - pads[None, :]                         # [n_sel, 4] sorted
    lab_sorted = lab_sel[order]
    numer = Sreal[np.arange(n_sel), lab_sorted]
    denom = Sreal.sum(axis=1)
    per = -np.log(numer / denom)
    loss = np.float32(per.sum() / max(n_sel, 1))
    return np.asarray(loss, dtype=np.float32)
